# revision 11
# baseline (speedup 1.0000x reference)
"""Trainium2 Bass kernel for nn_ExpandingLinear.

Computation (see reference):
    x_exp = concat([x, x[:, p0] * v0, x_exp1[:, p1] * v1], axis=1)   # [B, 2176]
    W     = scatter_add(weight_vals at [weight_rows, weight_cols])    # [2048, 2176]
    b     = scatter_add(bias_vals at bias_idx)                        # [2048]
    out   = x_exp @ W.T + b                                           # [B, 2048]

Sharding: data-parallel over the batch dim across 8 NeuronCores (1024 rows
per core); the weight/bias/embed parameters are replicated.

Host-side prep is limited to sharding and parameter/layout preparation
(batch split, x transpose, COO->dense weight densification, embed parent-chain
resolution); all O(batch) compute — the embed feature construction, the
full dense matmul and the bias add — runs on device.

Device kernel (per core):
  - xt [2048, 1024] (x shard, feature-major) streamed in as [128,128] k-tiles
  - wt [2176, 2048] (W^T) resident in SBUF
  - 128 embed features built on device: indirect-DMA row gather from xt
    + per-partition scale; forms contraction k-tile 16
  - out[m*128:(m+1)*128, :] = sum_k xt_tile[k,m].T @ wt_tile[k] + bias
    (PE matmul in float32r, PSUM fp32 accumulation over 17 k-tiles)
"""

import numpy as np
from contextlib import ExitStack

OUT = 2048
IN_BASE = 2048
N_EMBED = 64
IN_TOT = IN_BASE + 2 * N_EMBED  # 2176
BATCH = 8192
N_CORES = 8
B_CORE = BATCH // N_CORES       # 1024
P = 128
K_TILES = IN_TOT // P           # 17
M_TILES = B_CORE // P           # 8
N_SPLIT = 4                     # 2048 out cols in 4 x 512 (one PSUM bank each)

_CACHED = {}


def _build_nc():
    import concourse.bass as bass
    import concourse.mybir as mybir
    import concourse.tile as tile
    from concourse import bacc

    f32 = mybir.dt.float32
    f32r = mybir.dt.float32r
    i32 = mybir.dt.int32

    nc = bacc.Bacc("TRN2", target_bir_lowering=False, debug=False,
                   num_devices=N_CORES)

    xt = nc.dram_tensor("xt", [B_CORE, (K_TILES - 1) * P], f32r,
                        kind="ExternalInput")  # pre-tiled: row m*128+p
    xg = nc.dram_tensor("xg", [IN_BASE, B_CORE], f32r, kind="ExternalInput")
    wt = nc.dram_tensor("wt", [IN_TOT, OUT], f32r, kind="ExternalInput")
    bias = nc.dram_tensor("bias", [P, OUT], f32, kind="ExternalInput")
    emb_q = nc.dram_tensor("emb_q", [P, 1], i32, kind="ExternalInput")
    emb_a = nc.dram_tensor("emb_a", [P, 1], f32, kind="ExternalInput")
    out = nc.dram_tensor("out", [B_CORE, OUT], f32, kind="ExternalOutput")

    NW = 512  # wt stream chunk width == one fp32 PSUM bank

    with tile.TileContext(nc) as tc:
        with ExitStack() as ctx:
            wt_pool = ctx.enter_context(tc.tile_pool(name="wt", bufs=9))
            xt_pool = ctx.enter_context(tc.tile_pool(name="xt", bufs=M_TILES))
            small_pool = ctx.enter_context(tc.tile_pool(name="small", bufs=1))
            out_pool = ctx.enter_context(tc.tile_pool(name="out", bufs=4))
            psum_pool = ctx.enter_context(
                tc.tile_pool(name="psum", bufs=8, space="PSUM"))

            # gpsimd SWDGE queue order matters (FIFO): embed params + first
            # xt tile + gather first, remaining xt tiles, bias last (only
            # needed at first evac). Both HWDGE queues stream wt chunks.
            q_t = small_pool.tile([P, 1], i32, tag="q")
            nc.gpsimd.dma_start(out=q_t[:], in_=emb_q.ap())
            a_t = small_pool.tile([P, 1], f32, tag="a")
            nc.gpsimd.dma_start(out=a_t[:], in_=emb_a.ap())

            xt_tiles = []

            def load_xt(m):
                xt_m = xt_pool.tile([P, (K_TILES - 1) * P], f32r, tag="xt",
                                    name=f"xt_m{m}")
                nc.gpsimd.dma_start(
                    out=xt_m[:], in_=xt.ap()[m * P:(m + 1) * P, :])
                xt_tiles.append(xt_m)

            load_xt(0)

            # embed features: gather parent rows of xt, scale by alpha.
            # partition j = expanded feature 2048+j; k-tile 16 of x_exp^T.
            emb_raw = small_pool.tile([P, B_CORE], f32r, tag="emb_raw")
            nc.gpsimd.indirect_dma_start(
                out=emb_raw[:],
                out_offset=None,
                in_=xg.ap(),
                in_offset=bass.IndirectOffsetOnAxis(ap=q_t[:, 0:1], axis=0),
            )
            emb_t = small_pool.tile([P, B_CORE], f32r, tag="emb")
            nc.vector.tensor_scalar_mul(
                emb_t[:], emb_raw[:].bitcast(f32), a_t[:, 0:1])

            for m in range(1, M_TILES):
                load_xt(m)

            bias_t = small_pool.tile([P, OUT], f32, tag="bias")
            nc.gpsimd.dma_start(out=bias_t[:], in_=bias.ap())

            # stream W^T n-major in [128, CK*512] chunks alternating across
            # both HWDGE queues; 8 single-bank PSUM accumulators = all 8
            # m-tiles in flight per n, so PE starts as soon as chunk 0 lands.
            CK = 2
            wt_ap3 = wt.ap().rearrange("(k p) n -> p k n", p=P)  # [128,17,2048]
            k_chunks = [(k0, min(CK, K_TILES - k0))
                        for k0 in range(0, K_TILES, CK)]
            dma_engines = [nc.sync, nc.scalar]
            ci = 0

            def load_wt_chunk(n, k0, klen):
                wck = wt_pool.tile([P, CK * NW], f32r, tag="wck",
                                   name=f"wck_n{n}_k{k0}")
                nc_dma = dma_engines[load_wt_chunk.ci % 2]
                load_wt_chunk.ci += 1
                nc_dma.dma_start(
                    out=wck[:, :klen * NW].rearrange(
                        "p (k c) -> p k c", k=klen),
                    in_=wt_ap3[:, k0:k0 + klen, n * NW:(n + 1) * NW])
                return wck

            load_wt_chunk.ci = 0

            def mm(psum, k, m, wck, kk):
                if k < K_TILES - 1:
                    lhsT = xt_tiles[m][:, k * P:(k + 1) * P]
                else:
                    lhsT = emb_t[:, m * P:(m + 1) * P]
                nc.tensor.matmul(
                    psum[:],
                    lhsT=lhsT,
                    rhs=wck[:, kk * NW:(kk + 1) * NW],
                    start=(k == 0),
                    stop=(k == K_TILES - 1),
                )

            for n in range(N_SPLIT):
                psums = [psum_pool.tile([P, NW], f32, tag="ps",
                                        name=f"ps_n{n}_m{m}")
                         for m in range(M_TILES)]
                if n == 0:
                    # round 0: xt tiles are still streaming in at ~1.4us
                    # cadence — iterate m-outer so PE chases xt arrivals.
                    wcks = [load_wt_chunk(n, k0, klen)
                            for k0, klen in k_chunks]
                    for m in range(M_TILES):
                        for (k0, klen), wck in zip(k_chunks, wcks):
                            for kk in range(klen):
                                mm(psums[m], k0 + kk, m, wck, kk)
                else:
                    # steady state: k-outer m-inner chases the wt stream.
                    for k0, klen in k_chunks:
                        wck = load_wt_chunk(n, k0, klen)
                        for kk in range(klen):
                            for m in range(M_TILES):
                                mm(psums[m], k0 + kk, m, wck, kk)
                for m in range(M_TILES):
                    ot = out_pool.tile([P, NW], f32, tag="ot")
                    nc.vector.tensor_add(
                        ot[:], psums[m][:], bias_t[:, n * NW:(n + 1) * NW])
                    nc.gpsimd.dma_start(
                        out=out.ap()[m * P:(m + 1) * P, n * NW:(n + 1) * NW],
                        in_=ot[:])

    nc.compile()
    return nc


def _host_prep(inputs):
    x = np.ascontiguousarray(np.asarray(inputs["x"], dtype=np.float32))
    wv = np.asarray(inputs["weight_vals"], dtype=np.float32)
    wr = np.asarray(inputs["weight_rows"]).astype(np.int64)
    wc = np.asarray(inputs["weight_cols"]).astype(np.int64)
    bv = np.asarray(inputs["bias_vals"], dtype=np.float32)
    bi = np.asarray(inputs["bias_idx"]).astype(np.int64)
    e0v = np.asarray(inputs["embed0_vals"], dtype=np.float32)
    e0p = np.asarray(inputs["embed0_parents"]).astype(np.int64)
    e1v = np.asarray(inputs["embed1_vals"], dtype=np.float32)
    e1p = np.asarray(inputs["embed1_parents"]).astype(np.int64)

    # dense W^T [IN_TOT, OUT] (coalesce: duplicates sum)
    wt = np.bincount(wc * OUT + wr, weights=wv,
                     minlength=IN_TOT * OUT).reshape(IN_TOT, OUT)
    wt = np.ascontiguousarray(wt.astype(np.float32))

    b = np.bincount(bi, weights=bv, minlength=OUT).astype(np.float32)
    bias_bcast = np.ascontiguousarray(
        np.broadcast_to(b[None, :], (P, OUT)).astype(np.float32))

    # resolve embed parent chains to direct (row-in-x, multiplier) pairs
    q = np.empty(2 * N_EMBED, dtype=np.int32)
    a = np.empty(2 * N_EMBED, dtype=np.float32)
    q[:N_EMBED] = e0p
    a[:N_EMBED] = e0v
    for j in range(N_EMBED):
        p = int(e1p[j])
        if p < IN_BASE:
            q[N_EMBED + j] = p
            a[N_EMBED + j] = e1v[j]
        else:
            t = p - IN_BASE
            q[N_EMBED + j] = e0p[t]
            a[N_EMBED + j] = e1v[j] * e0v[t]

    xts = []
    xgs = []
    for i in range(N_CORES):
        xs = x[i * B_CORE:(i + 1) * B_CORE]
        # SBUF-tiled layout: row m*128+p, col k*128+f  ==  xs[m*128+f, k*128+p]
        xts.append(np.ascontiguousarray(
            xs.reshape(M_TILES, P, K_TILES - 1, P)
              .transpose(0, 3, 2, 1).reshape(B_CORE, (K_TILES - 1) * P)))
        xgs.append(np.ascontiguousarray(xs.T))
    return xts, xgs, wt, bias_bcast, q.reshape(P, 1), a.reshape(P, 1)


def kernel(**inputs) -> np.ndarray:
    import time
    from concourse.bass_utils import run_bass_kernel_spmd

    if "nc" not in _CACHED:
        _CACHED["nc"] = _build_nc()
    nc = _CACHED["nc"]

    xts, xgs, wt, bias_bcast, q, a = _host_prep(inputs)
    in_maps = [
        dict(xt=xts[i], xg=xgs[i], wt=wt, bias=bias_bcast, emb_q=q, emb_a=a)
        for i in range(N_CORES)
    ]
    res = None
    last_exc = None
    for attempt in range(3):
        try:
            res = run_bass_kernel_spmd(nc, in_maps,
                                       core_ids=list(range(N_CORES)))
            break
        except Exception as e:  # transient device/runtime hiccups
            last_exc = e
            time.sleep(2.0)
    if res is None:
        raise last_exc
    out = np.concatenate([res.results[i]["out"] for i in range(N_CORES)],
                         axis=0)
    return np.ascontiguousarray(out.astype(np.float32))


# revision 12
# speedup vs baseline: 1.0148x; 1.0148x over previous
"""Trainium2 Bass kernel for nn_ExpandingLinear.

Computation (see reference):
    x_exp = concat([x, x[:, p0] * v0, x_exp1[:, p1] * v1], axis=1)   # [B, 2176]
    W     = scatter_add(weight_vals at [weight_rows, weight_cols])    # [2048, 2176]
    b     = scatter_add(bias_vals at bias_idx)                        # [2048]
    out   = x_exp @ W.T + b                                           # [B, 2048]

Sharding: data-parallel over the batch dim across 8 NeuronCores (1024 rows
per core); the weight/bias/embed parameters are replicated.

Host-side prep is limited to sharding and parameter/layout preparation
(batch split, x transpose, COO->dense weight densification, embed parent-chain
resolution); all O(batch) compute — the embed feature construction, the
full dense matmul and the bias add — runs on device.

Device kernel (per core):
  - xt [2048, 1024] (x shard, feature-major) streamed in as [128,128] k-tiles
  - wt [2176, 2048] (W^T) resident in SBUF
  - 128 embed features built on device: indirect-DMA row gather from xt
    + per-partition scale; forms contraction k-tile 16
  - out[m*128:(m+1)*128, :] = sum_k xt_tile[k,m].T @ wt_tile[k] + bias
    (PE matmul in float32r, PSUM fp32 accumulation over 17 k-tiles)
"""

import numpy as np
from contextlib import ExitStack

OUT = 2048
IN_BASE = 2048
N_EMBED = 64
IN_TOT = IN_BASE + 2 * N_EMBED  # 2176
BATCH = 8192
N_CORES = 8
B_CORE = BATCH // N_CORES       # 1024
P = 128
K_TILES = IN_TOT // P           # 17
M_TILES = B_CORE // P           # 8
N_SPLIT = 4                     # 2048 out cols in 4 x 512 (one PSUM bank each)

_CACHED = {}


def _build_nc():
    import concourse.bass as bass
    import concourse.mybir as mybir
    import concourse.tile as tile
    from concourse import bacc

    f32 = mybir.dt.float32
    f32r = mybir.dt.float32r
    i32 = mybir.dt.int32

    nc = bacc.Bacc("TRN2", target_bir_lowering=False, debug=False,
                   num_devices=N_CORES)

    xt = nc.dram_tensor("xt", [B_CORE, (K_TILES - 1) * P], f32r,
                        kind="ExternalInput")  # pre-tiled: row m*128+p
    xg = nc.dram_tensor("xg", [IN_BASE, B_CORE], f32r, kind="ExternalInput")
    wt = nc.dram_tensor("wt", [IN_TOT, OUT], f32r, kind="ExternalInput")
    bias = nc.dram_tensor("bias", [P, OUT], f32, kind="ExternalInput")
    emb_q = nc.dram_tensor("emb_q", [P, 1], i32, kind="ExternalInput")
    emb_a = nc.dram_tensor("emb_a", [P, 1], f32, kind="ExternalInput")
    out = nc.dram_tensor("out", [B_CORE, OUT], f32, kind="ExternalOutput")

    NW = 512  # wt stream chunk width == one fp32 PSUM bank

    with tile.TileContext(nc) as tc:
        with ExitStack() as ctx:
            wt_pool = ctx.enter_context(tc.tile_pool(name="wt", bufs=11))
            xt_pool = ctx.enter_context(tc.tile_pool(name="xt", bufs=M_TILES))
            small_pool = ctx.enter_context(tc.tile_pool(name="small", bufs=1))
            out_pool = ctx.enter_context(tc.tile_pool(name="out", bufs=4))
            psum_pool = ctx.enter_context(
                tc.tile_pool(name="psum", bufs=8, space="PSUM"))

            # gpsimd SWDGE queue order matters (FIFO): embed params + first
            # xt tile + gather first, remaining xt tiles, bias last (only
            # needed at first evac). Both HWDGE queues stream wt chunks.
            q_t = small_pool.tile([P, 1], i32, tag="q")
            nc.gpsimd.dma_start(out=q_t[:], in_=emb_q.ap())
            a_t = small_pool.tile([P, 1], f32, tag="a")
            nc.gpsimd.dma_start(out=a_t[:], in_=emb_a.ap())

            xt_tiles = []

            def load_xt(m):
                xt_m = xt_pool.tile([P, (K_TILES - 1) * P], f32r, tag="xt",
                                    name=f"xt_m{m}")
                nc.gpsimd.dma_start(
                    out=xt_m[:], in_=xt.ap()[m * P:(m + 1) * P, :])
                xt_tiles.append(xt_m)

            for m in range(M_TILES):
                load_xt(m)

            # embed features: gather parent rows of x (feature-major copy),
            # scale by alpha. partition j = expanded feature 2048+j; forms
            # k-tile 16 of x_exp^T. Emitted after the xt loads: its Q7-side
            # wait would otherwise stall the SWDGE ring behind it.
            emb_raw = small_pool.tile([P, B_CORE], f32r, tag="emb_raw")
            nc.gpsimd.indirect_dma_start(
                out=emb_raw[:],
                out_offset=None,
                in_=xg.ap(),
                in_offset=bass.IndirectOffsetOnAxis(ap=q_t[:, 0:1], axis=0),
            )
            emb_t = small_pool.tile([P, B_CORE], f32r, tag="emb")
            nc.vector.tensor_scalar_mul(
                emb_t[:], emb_raw[:].bitcast(f32), a_t[:, 0:1])

            bias_t = small_pool.tile([P, OUT], f32, tag="bias")
            nc.gpsimd.dma_start(out=bias_t[:], in_=bias.ap())

            # stream W^T n-major in [128, CK*512] chunks alternating across
            # both HWDGE queues; 8 single-bank PSUM accumulators = all 8
            # m-tiles in flight per n, so PE starts as soon as chunk 0 lands.
            CK = 2
            wt_ap3 = wt.ap().rearrange("(k p) n -> p k n", p=P)  # [128,17,2048]
            k_chunks = [(k0, min(CK, K_TILES - k0))
                        for k0 in range(0, K_TILES, CK)]
            dma_engines = [nc.sync, nc.scalar]
            ci = 0

            def load_wt_chunk(n, k0, klen):
                wck = wt_pool.tile([P, CK * NW], f32r, tag="wck",
                                   name=f"wck_n{n}_k{k0}")
                nc_dma = dma_engines[load_wt_chunk.ci % 2]
                load_wt_chunk.ci += 1
                nc_dma.dma_start(
                    out=wck[:, :klen * NW].rearrange(
                        "p (k c) -> p k c", k=klen),
                    in_=wt_ap3[:, k0:k0 + klen, n * NW:(n + 1) * NW])
                return wck

            load_wt_chunk.ci = 0

            def mm(psum, k, m, wck, kk):
                if k < K_TILES - 1:
                    lhsT = xt_tiles[m][:, k * P:(k + 1) * P]
                else:
                    lhsT = emb_t[:, m * P:(m + 1) * P]
                nc.tensor.matmul(
                    psum[:],
                    lhsT=lhsT,
                    rhs=wck[:, kk * NW:(kk + 1) * NW],
                    start=(k == 0),
                    stop=(k == K_TILES - 1),
                )

            for n in range(N_SPLIT):
                psums = [psum_pool.tile([P, NW], f32, tag="ps",
                                        name=f"ps_n{n}_m{m}")
                         for m in range(M_TILES)]
                if n == 0:
                    # round 0: xt tiles are still streaming in at ~1.4us
                    # cadence — iterate m-outer so PE chases xt arrivals.
                    wcks = [load_wt_chunk(n, k0, klen)
                            for k0, klen in k_chunks]
                    for m in range(M_TILES):
                        for (k0, klen), wck in zip(k_chunks, wcks):
                            for kk in range(klen):
                                mm(psums[m], k0 + kk, m, wck, kk)
                else:
                    # steady state: k-outer m-inner chases the wt stream.
                    for k0, klen in k_chunks:
                        wck = load_wt_chunk(n, k0, klen)
                        for kk in range(klen):
                            for m in range(M_TILES):
                                mm(psums[m], k0 + kk, m, wck, kk)
                for m in range(M_TILES):
                    ot = out_pool.tile([P, NW], f32, tag="ot")
                    nc.vector.tensor_add(
                        ot[:], psums[m][:], bias_t[:, n * NW:(n + 1) * NW])
                    nc.gpsimd.dma_start(
                        out=out.ap()[m * P:(m + 1) * P, n * NW:(n + 1) * NW],
                        in_=ot[:])

    nc.compile()
    return nc


def _host_prep(inputs):
    x = np.ascontiguousarray(np.asarray(inputs["x"], dtype=np.float32))
    wv = np.asarray(inputs["weight_vals"], dtype=np.float32)
    wr = np.asarray(inputs["weight_rows"]).astype(np.int64)
    wc = np.asarray(inputs["weight_cols"]).astype(np.int64)
    bv = np.asarray(inputs["bias_vals"], dtype=np.float32)
    bi = np.asarray(inputs["bias_idx"]).astype(np.int64)
    e0v = np.asarray(inputs["embed0_vals"], dtype=np.float32)
    e0p = np.asarray(inputs["embed0_parents"]).astype(np.int64)
    e1v = np.asarray(inputs["embed1_vals"], dtype=np.float32)
    e1p = np.asarray(inputs["embed1_parents"]).astype(np.int64)

    # dense W^T [IN_TOT, OUT] (coalesce: duplicates sum)
    wt = np.bincount(wc * OUT + wr, weights=wv,
                     minlength=IN_TOT * OUT).reshape(IN_TOT, OUT)
    wt = np.ascontiguousarray(wt.astype(np.float32))

    b = np.bincount(bi, weights=bv, minlength=OUT).astype(np.float32)
    bias_bcast = np.ascontiguousarray(
        np.broadcast_to(b[None, :], (P, OUT)).astype(np.float32))

    # resolve embed parent chains to direct (row-in-x, multiplier) pairs
    q = np.empty(2 * N_EMBED, dtype=np.int32)
    a = np.empty(2 * N_EMBED, dtype=np.float32)
    q[:N_EMBED] = e0p
    a[:N_EMBED] = e0v
    for j in range(N_EMBED):
        p = int(e1p[j])
        if p < IN_BASE:
            q[N_EMBED + j] = p
            a[N_EMBED + j] = e1v[j]
        else:
            t = p - IN_BASE
            q[N_EMBED + j] = e0p[t]
            a[N_EMBED + j] = e1v[j] * e0v[t]

    xts = []
    xgs = []
    for i in range(N_CORES):
        xs = x[i * B_CORE:(i + 1) * B_CORE]
        # SBUF-tiled layout: row m*128+p, col k*128+f  ==  xs[m*128+f, k*128+p]
        xts.append(np.ascontiguousarray(
            xs.reshape(M_TILES, P, K_TILES - 1, P)
              .transpose(0, 3, 2, 1).reshape(B_CORE, (K_TILES - 1) * P)))
        xgs.append(np.ascontiguousarray(xs.T))
    return xts, xgs, wt, bias_bcast, q.reshape(P, 1), a.reshape(P, 1)


def kernel(**inputs) -> np.ndarray:
    import time
    from concourse.bass_utils import run_bass_kernel_spmd

    if "nc" not in _CACHED:
        _CACHED["nc"] = _build_nc()
    nc = _CACHED["nc"]

    xts, xgs, wt, bias_bcast, q, a = _host_prep(inputs)
    in_maps = [
        dict(xt=xts[i], xg=xgs[i], wt=wt, bias=bias_bcast, emb_q=q, emb_a=a)
        for i in range(N_CORES)
    ]
    res = None
    last_exc = None
    for attempt in range(3):
        try:
            res = run_bass_kernel_spmd(nc, in_maps,
                                       core_ids=list(range(N_CORES)))
            break
        except Exception as e:  # transient device/runtime hiccups
            last_exc = e
            time.sleep(2.0)
    if res is None:
        raise last_exc
    out = np.concatenate([res.results[i]["out"] for i in range(N_CORES)],
                         axis=0)
    return np.ascontiguousarray(out.astype(np.float32))
